# revision 9
# baseline (speedup 1.0000x reference)
"""Trainium2 Bass kernel for an 8-batch image-conditioned decoder layer.

Strategy: pure data-parallel over the batch — core c computes batch element c
end-to-end.  v2: per-seq-tile (qt) software-pipelined attention with vocab
chunks front-loaded as PE filler, exact 32000-wide vocab projection (no pad),
g2/b2 folded into Wp/bp on the host, in-place PSUM masking.

All matmuls run in bf16 with fp32 PSUM accumulation.
"""

import os
import sys

for _p in ("/opt/trn_rl_repo", "/root/.axon_site/_ro/trn_rl_repo"):
    if os.path.isdir(_p) and _p not in sys.path:
        sys.path.append(_p)

import numpy as np
import ml_dtypes

BF16 = ml_dtypes.bfloat16

# Problem dims (hardcoded per spec)
V, D, DI, S, B, NI = 32000, 1024, 768, 512, 8, 197
EPS = 1e-5
P = 128
ST = S // P          # 4 seq tiles
DT = D // P          # 8 model-dim tiles
DIT = DI // P        # 6 image-dim tiles
NI2 = NI - P         # 69 rows in second image tile
CN = 512             # vocab chunk width
NFULL = V // CN      # 62 full chunks
CLAST = V - NFULL * CN   # 256
NCHUNK = NFULL + 1   # 63
N_CORES = 8
SCALE = 1.0 / float(np.sqrt(np.float32(D)))

_CACHE = {}
LAST_RESULTS = None


def _build_program():
    import concourse.bacc as bacc
    import concourse.bass as bass
    import concourse.mybir as mybir
    from concourse.masks import make_identity
    from concourse.tile import TileContext

    f32 = mybir.dt.float32
    bf16 = mybir.dt.bfloat16
    i32 = mybir.dt.int32
    X = mybir.AxisListType.X
    ALU = mybir.AluOpType
    ACT_F = mybir.ActivationFunctionType

    nc = bacc.Bacc("TRN2", target_bir_lowering=False, debug=False,
                   num_devices=N_CORES)

    # ---- I/O ----
    h_tok = nc.dram_tensor("tok", [S], i32, kind="ExternalInput")
    h_table = nc.dram_tensor("table", [V, D], bf16, kind="ExternalInput")
    h_pos = nc.dram_tensor("pos", [S, D], bf16, kind="ExternalInput")
    h_img = nc.dram_tensor("img_t", [P, DIT, NI], bf16, kind="ExternalInput")
    h_wq1 = nc.dram_tensor("wq1", [P, DT, D], bf16, kind="ExternalInput")
    h_wk1 = nc.dram_tensor("wk1", [P, DT, D], bf16, kind="ExternalInput")
    h_wv1 = nc.dram_tensor("wv1", [P, DT, D], bf16, kind="ExternalInput")
    h_wq2 = nc.dram_tensor("wq2", [P, DT, D], bf16, kind="ExternalInput")
    h_wk2 = nc.dram_tensor("wk2", [P, DIT, D], bf16, kind="ExternalInput")
    h_wv2 = nc.dram_tensor("wv2", [P, DIT, D], bf16, kind="ExternalInput")
    h_wp = nc.dram_tensor("wp", [NFULL, P, DT, CN], bf16, kind="ExternalInput")
    h_wpl = nc.dram_tensor("wpl", [P, DT, CLAST], bf16, kind="ExternalInput")
    h_bq1 = nc.dram_tensor("bq1", [P, DT], f32, kind="ExternalInput")
    h_bk1 = nc.dram_tensor("bk1", [P, DT], f32, kind="ExternalInput")
    h_bq2 = nc.dram_tensor("bq2", [P, DT], f32, kind="ExternalInput")
    h_bk2 = nc.dram_tensor("bk2", [P, DT], f32, kind="ExternalInput")
    h_bv1 = nc.dram_tensor("bv1", [D], bf16, kind="ExternalInput")
    h_bv2 = nc.dram_tensor("bv2", [D], bf16, kind="ExternalInput")
    h_bp = nc.dram_tensor("bp", [V], bf16, kind="ExternalInput")
    h_g1 = nc.dram_tensor("g1", [D], bf16, kind="ExternalInput")
    h_b1 = nc.dram_tensor("b1", [D], bf16, kind="ExternalInput")
    h_out = nc.dram_tensor("out", [S, V], bf16, kind="ExternalOutput")

    def bcast(handle, n, offset=0):
        ap = handle[:]
        return bass.AP(tensor=ap.tensor, offset=offset, ap=[[0, P], [1, n]])

    with TileContext(nc) as tc:
        import contextlib
        ctx = contextlib.ExitStack()
        with ctx:
            const = ctx.enter_context(tc.tile_pool(name="const", bufs=1))
            posp = ctx.enter_context(tc.tile_pool(name="posp", bufs=2))
            x0b_p = ctx.enter_context(tc.tile_pool(name="x0b", bufs=4))
            xt_p = ctx.enter_context(tc.tile_pool(name="xt", bufs=2))
            qt_p = ctx.enter_context(tc.tile_pool(name="qtp", bufs=2))
            kt_p = ctx.enter_context(tc.tile_pool(name="ktp", bufs=4))
            vt_p = ctx.enter_context(tc.tile_pool(name="vtp", bufs=4))
            k2t_p = ctx.enter_context(tc.tile_pool(name="k2t", bufs=1))
            v2t_p = ctx.enter_context(tc.tile_pool(name="v2t", bufs=1))
            pb_p = ctx.enter_context(tc.tile_pool(name="pb", bufs=2))
            pt_p = ctx.enter_context(tc.tile_pool(name="pt", bufs=2))
            x1b_p = ctx.enter_context(tc.tile_pool(name="x1b", bufs=2))
            x1t_p = ctx.enter_context(tc.tile_pool(name="x1t", bufs=2))
            q2t_p = ctx.enter_context(tc.tile_pool(name="q2t", bufs=2))
            p2b_p = ctx.enter_context(tc.tile_pool(name="p2b", bufs=2))
            pt2_p = ctx.enter_context(tc.tile_pool(name="pt2", bufs=2))
            x2n_p = ctx.enter_context(tc.tile_pool(name="x2n", bufs=2))
            x2t_p = ctx.enter_context(tc.tile_pool(name="x2t", bufs=4))
            xpre_p = ctx.enter_context(tc.tile_pool(name="xpre", bufs=2))
            stat_p = ctx.enter_context(tc.tile_pool(name="stat", bufs=4))
            wts_p = ctx.enter_context(tc.tile_pool(name="wts", bufs=5))
            wp_p = ctx.enter_context(tc.tile_pool(name="wpp", bufs=3))
            bp_p = ctx.enter_context(tc.tile_pool(name="bpp", bufs=2))
            osb_p = ctx.enter_context(tc.tile_pool(name="osb", bufs=6))
            ps = ctx.enter_context(tc.tile_pool(name="ps", bufs=8, space="PSUM"))

            # ---- t=0 DMA kickoff across queues ----
            tok_sb = const.tile([P, ST], i32)
            nc.sync.dma_start(out=tok_sb,
                              in_=h_tok[:].rearrange("(a p) -> p a", p=P))
            img_sb = const.tile([P, DIT, NI], bf16)
            nc.scalar.dma_start(out=img_sb, in_=h_img[:])
            wk2_sb = wts_p.tile([P, DIT, D], bf16, tag="wts", name="wk2")
            nc.scalar.dma_start(out=wk2_sb, in_=h_wk2[:])
            wv2_sb = wts_p.tile([P, DIT, D], bf16, tag="wts", name="wv2")
            nc.scalar.dma_start(out=wv2_sb, in_=h_wv2[:])
            wq1_sb = wts_p.tile([P, DT, D], bf16, tag="wts", name="wq1")
            nc.sync.dma_start(out=wq1_sb, in_=h_wq1[:])
            wv1_sb = wts_p.tile([P, DT, D], bf16, tag="wts", name="wv1")
            nc.scalar.dma_start(out=wv1_sb, in_=h_wv1[:])
            wk1_sb = wts_p.tile([P, DT, D], bf16, tag="wts", name="wk1")
            nc.sync.dma_start(out=wk1_sb, in_=h_wk1[:])

            epst = const.tile([P, 1], f32)
            nc.vector.memset(epst, EPS)
            bq1s = const.tile([P, DT], f32)
            bk1s = const.tile([P, DT], f32)
            bq2s = const.tile([P, DT], f32)
            bk2s = const.tile([P, DT], f32)
            for t, h in ((bk2s, h_bk2), (bq1s, h_bq1), (bk1s, h_bk1),
                         (bq2s, h_bq2)):
                nc.sync.dma_start(out=t, in_=h[:])
            g1b = const.tile([P, D], bf16)
            b1b = const.tile([P, D], bf16)
            bv1b = const.tile([P, D], bf16)
            bv2b = const.tile([P, D], bf16)
            for t, h in ((bv2b, h_bv2), (g1b, h_g1), (b1b, h_b1),
                         (bv1b, h_bv1)):
                nc.sync.dma_start(out=t, in_=bcast(h, D))

            # embedding gather (gpsimd) + pos tiles (sync)
            x0b = [x0b_p.tile([P, D], bf16, tag="x0b", name=f"x0b{q}")
                   for q in range(ST)]
            for q in range(ST):
                nc.gpsimd.indirect_dma_start(
                    out=x0b[q], out_offset=None, in_=h_table[:],
                    in_offset=bass.IndirectOffsetOnAxis(ap=tok_sb[:, q:q + 1],
                                                        axis=0))
            ident = const.tile([P, P], bf16)
            make_identity(nc, ident)
            trimask = const.tile([P, P], f32)
            nc.gpsimd.memset(trimask, 0.0)
            nc.gpsimd.affine_select(
                out=trimask, in_=trimask, compare_op=ALU.is_ge, fill=-1e10,
                base=0, pattern=[[-1, P]], channel_multiplier=1)

            # ---- PE warmup: K2T / V2t (depend only on img weights) ----
            K2T = k2t_p.tile([P, DT, NI], bf16, tag="k2t")
            for m in range(DT):
                pm = ps.tile([P, 512], f32, tag="ps", name="k2ps")
                for k in range(DIT):
                    nc.tensor.matmul(pm[:, :NI],
                                     lhsT=wk2_sb[:, k, m * P:(m + 1) * P],
                                     rhs=img_sb[:, k, :],
                                     start=(k == 0), stop=(k == DIT - 1))
                nc.scalar.activation(out=K2T[:, m, :], in_=pm[:, :NI],
                                     func=ACT_F.Identity,
                                     bias=bk2s[:, m:m + 1], scale=1.0)
            # wq2 reuses wk2's pool slot; emit after K2T (wk2's readers) exist
            wq2_sb = wts_p.tile([P, DT, D], bf16, tag="wts", name="wq2")
            nc.scalar.dma_start(out=wq2_sb, in_=h_wq2[:])
            V2t = v2t_p.tile([P, 2, D], bf16, tag="v2t")
            for a in range(2):
                pa = P if a == 0 else NI2
                for nh in range(2):
                    pm = ps.tile([P, 512], f32, tag="ps", name="v2ps")
                    for k in range(DIT):
                        nc.tensor.matmul(
                            pm[:pa, :], lhsT=img_sb[:, k, a * P:a * P + pa],
                            rhs=wv2_sb[:, k, nh * 512:(nh + 1) * 512],
                            start=(k == 0), stop=(k == DIT - 1))
                    nc.vector.tensor_tensor(
                        out=V2t[:pa, a, nh * 512:(nh + 1) * 512], in0=pm[:pa, :],
                        in1=bv2b[:pa, nh * 512:(nh + 1) * 512], op=ALU.add)

            # ---- per-qt attention stages ----
            KT = [None] * ST
            Vt = [None] * ST
            x2T = [None] * ST
            rinv1 = const.tile([P, ST], f32)
            rinv2 = const.tile([P, ST], f32)
            Pb = [None] * ST
            P2b = [None] * ST
            QTq = [None] * ST
            Q2Tq = [None] * ST
            x1b = [None] * ST

            def stage_x0(q):
                post = posp.tile([P, D], bf16, tag="pos")
                nc.sync.dma_start(out=post, in_=h_pos[q * P:(q + 1) * P, :])
                nc.vector.tensor_tensor(out=x0b[q], in0=x0b[q], in1=post,
                                        op=ALU.add)

            def transpose8(src, dst, tagname):
                """src [P, D] bf16 (seq-part) -> dst [P, DT, P] bf16 (d-part)."""
                for db in range(DT):
                    tp = ps.tile([P, 512], bf16, tag="ps", name="tp")
                    nc.tensor.transpose(out=tp[:, :P],
                                        in_=src[:, db * P:(db + 1) * P],
                                        identity=ident)
                    nc.vector.tensor_copy(out=dst[:, db, :], in_=tp[:, :P])

            x0T_cur = [None] * ST

            def stage_qkv(q):
                x0T = xt_p.tile([P, DT, P], bf16, tag="xt", name=f"x0T{q}")
                transpose8(x0b[q], x0T, f"x0T{q}")
                x0T_cur[q] = x0T
                QT = qt_p.tile([P, DT, P], bf16, tag="qt", name=f"QT{q}")
                KTq = kt_p.tile([P, DT, P], bf16, tag="kt", name=f"KT{q}")
                for o, w_sb, b_sb in ((QT, wq1_sb, bq1s), (KTq, wk1_sb, bk1s)):
                    for m in range(DT):
                        pm = ps.tile([P, 512], f32, tag="ps", name="prps")
                        for k in range(DT):
                            nc.tensor.matmul(
                                pm[:, :P], lhsT=w_sb[:, k, m * P:(m + 1) * P],
                                rhs=x0T[:, k, :],
                                start=(k == 0), stop=(k == DT - 1))
                        nc.scalar.activation(out=o[:, m, :], in_=pm[:, :P],
                                             func=ACT_F.Identity,
                                             bias=b_sb[:, m:m + 1], scale=1.0)
                QTq[q] = QT
                KT[q] = KTq
                Vtq = vt_p.tile([P, D], bf16, tag="vt", name=f"Vt{q}")
                for nh in range(2):
                    pm = ps.tile([P, 512], f32, tag="ps", name="vps")
                    for k in range(DT):
                        nc.tensor.matmul(
                            pm, lhsT=x0T[:, k, :],
                            rhs=wv1_sb[:, k, nh * 512:(nh + 1) * 512],
                            start=(k == 0), stop=(k == DT - 1))
                    nc.vector.tensor_tensor(
                        out=Vtq[:, nh * 512:(nh + 1) * 512], in0=pm,
                        in1=bv1b[:, nh * 512:(nh + 1) * 512], op=ALU.add)
                Vt[q] = Vtq

            def stage_scores(q):
                width = (q + 1) * P
                pm = ps.tile([P, 512], f32, tag="ps", name=f"sc{q}")
                for kt in range(q + 1):
                    for k in range(DT):
                        nc.tensor.matmul(
                            pm[:, kt * P:(kt + 1) * P],
                            lhsT=QTq[q][:, k, :], rhs=KT[kt][:, k, :],
                            start=(k == 0), stop=(k == DT - 1))
                # in-place causal mask on the diagonal block (PSUM RMW)
                nc.vector.tensor_tensor(out=pm[:, q * P:width],
                                        in0=pm[:, q * P:width], in1=trimask,
                                        op=ALU.add)
                nmax = stat_p.tile([P, 1], f32, tag="nmax")
                nc.vector.reduce_max(nmax, pm[:, :width], axis=X, negate=True)
                Pbq = pb_p.tile([P, 512], bf16, tag="pb", name=f"pb{q}")
                rsum = stat_p.tile([P, 1], f32, tag="rsum")
                nc.scalar.activation(out=Pbq[:, :width], in_=pm[:, :width],
                                     func=ACT_F.Exp, bias=nmax, scale=1.0,
                                     accum_out=rsum)
                nc.vector.reciprocal(out=rinv1[:, q:q + 1], in_=rsum)
                Pb[q] = Pbq

            def layernorm(xpre, out_sl, affine):
                """xpre [P, D] f32 -> out_sl [P, D] bf16."""
                stats = stat_p.tile([P, 2, 6], f32, tag="bnst")
                for sg in range(2):
                    nc.vector.bn_stats(out=stats[:, sg, :],
                                       in_=xpre[:, sg * 512:(sg + 1) * 512])
                mv = stat_p.tile([P, 2], f32, tag="bnmv")
                nc.vector.bn_aggr(out=mv, in_=stats)
                rstd = stat_p.tile([P, 1], f32, tag="rstd")
                nc.scalar.activation(out=rstd, in_=mv[:, 1:2], func=ACT_F.Sqrt,
                                     bias=epst, scale=1.0)
                nc.vector.reciprocal(out=rstd, in_=rstd)
                nmr = stat_p.tile([P, 1], f32, tag="nmr")
                nc.vector.scalar_tensor_tensor(out=nmr, in0=mv[:, 0:1],
                                               scalar=-1.0, in1=rstd,
                                               op0=ALU.mult, op1=ALU.mult)
                if affine:
                    nc.scalar.activation(out=xpre, in_=xpre,
                                         func=ACT_F.Identity,
                                         bias=nmr, scale=rstd)
                    nc.vector.tensor_tensor(out=xpre, in0=xpre, in1=g1b,
                                            op=ALU.mult)
                    nc.vector.tensor_tensor(out=out_sl, in0=xpre, in1=b1b,
                                            op=ALU.add)
                else:
                    nc.scalar.activation(out=out_sl, in_=xpre,
                                         func=ACT_F.Identity,
                                         bias=nmr, scale=rstd)

            def stage_av(q):
                PT = pt_p.tile([P, ST, P], bf16, tag="pt", name=f"pt{q}")
                for kt in range(q + 1):
                    tp = ps.tile([P, 512], bf16, tag="ps", name="tp")
                    nc.tensor.transpose(out=tp[:, :P],
                                        in_=Pb[q][:, kt * P:(kt + 1) * P],
                                        identity=ident)
                    nc.vector.tensor_copy(out=PT[:, kt, :], in_=tp[:, :P])
                xpre = xpre_p.tile([P, D], f32, tag="xpre")
                for nh in range(2):
                    pm = ps.tile([P, 512], f32, tag="ps", name="avps")
                    for kt in range(q + 1):
                        nc.tensor.matmul(
                            pm, lhsT=PT[:, kt, :],
                            rhs=Vt[kt][:, nh * 512:(nh + 1) * 512],
                            start=(kt == 0), stop=(kt == q))
                    nc.vector.scalar_tensor_tensor(
                        out=xpre[:, nh * 512:(nh + 1) * 512], in0=pm,
                        scalar=rinv1[:, q:q + 1],
                        in1=x0b[q][:, nh * 512:(nh + 1) * 512],
                        op0=ALU.mult, op1=ALU.add)
                x1q = x1b_p.tile([P, D], bf16, tag="x1b", name=f"x1b{q}")
                layernorm(xpre, x1q, True)
                x1b[q] = x1q

            def stage_x1T_q2(q):
                x1T = x1t_p.tile([P, DT, P], bf16, tag="x1t", name=f"x1T{q}")
                transpose8(x1b[q], x1T, f"x1T{q}")
                Q2T = q2t_p.tile([P, DT, P], bf16, tag="q2t", name=f"Q2T{q}")
                for m in range(DT):
                    pm = ps.tile([P, 512], f32, tag="ps", name="q2ps")
                    for k in range(DT):
                        nc.tensor.matmul(
                            pm[:, :P], lhsT=wq2_sb[:, k, m * P:(m + 1) * P],
                            rhs=x1T[:, k, :],
                            start=(k == 0), stop=(k == DT - 1))
                    nc.scalar.activation(out=Q2T[:, m, :], in_=pm[:, :P],
                                         func=ACT_F.Identity,
                                         bias=bq2s[:, m:m + 1], scale=1.0)
                Q2Tq[q] = Q2T

            def stage_scores2(q):
                pm = ps.tile([P, 512], f32, tag="ps", name=f"sc2{q}")
                for k in range(DT):
                    nc.tensor.matmul(pm[:, :NI], lhsT=Q2Tq[q][:, k, :],
                                     rhs=K2T[:, k, :],
                                     start=(k == 0), stop=(k == DT - 1))
                nmax = stat_p.tile([P, 1], f32, tag="nmax")
                nc.vector.reduce_max(nmax, pm[:, :NI], axis=X, negate=True)
                P2bq = p2b_p.tile([P, NI], bf16, tag="p2b", name=f"p2b{q}")
                rsum = stat_p.tile([P, 1], f32, tag="rsum")
                nc.scalar.activation(out=P2bq, in_=pm[:, :NI],
                                     func=ACT_F.Exp, bias=nmax, scale=1.0,
                                     accum_out=rsum)
                nc.vector.reciprocal(out=rinv2[:, q:q + 1], in_=rsum)
                P2b[q] = P2bq

            def stage_av2(q):
                PT2 = pt2_p.tile([P, 2, P], bf16, tag="pt2", name=f"pt2{q}")
                tp = ps.tile([P, 512], bf16, tag="ps", name="tp")
                nc.tensor.transpose(out=tp[:, :P], in_=P2b[q][:, :P],
                                    identity=ident)
                nc.vector.tensor_copy(out=PT2[:, 0, :], in_=tp[:, :P])
                tp = ps.tile([P, 512], bf16, tag="ps", name="tp")
                nc.tensor.transpose(out=tp[:NI2, :P], in_=P2b[q][:, P:NI],
                                    identity=ident)
                nc.vector.tensor_copy(out=PT2[:NI2, 1, :], in_=tp[:NI2, :P])
                xpre = xpre_p.tile([P, D], f32, tag="xpre")
                for nh in range(2):
                    pm = ps.tile([P, 512], f32, tag="ps", name="av2ps")
                    nc.tensor.matmul(pm, lhsT=PT2[:, 0, :],
                                     rhs=V2t[:, 0, nh * 512:(nh + 1) * 512],
                                     start=True, stop=False)
                    nc.tensor.matmul(pm, lhsT=PT2[:NI2, 1, :],
                                     rhs=V2t[:NI2, 1, nh * 512:(nh + 1) * 512],
                                     start=False, stop=True)
                    nc.vector.scalar_tensor_tensor(
                        out=xpre[:, nh * 512:(nh + 1) * 512], in0=pm,
                        scalar=rinv2[:, q:q + 1],
                        in1=x1b[q][:, nh * 512:(nh + 1) * 512],
                        op0=ALU.mult, op1=ALU.add)
                x2n = x2n_p.tile([P, D], bf16, tag="x2n", name=f"x2n{q}")
                layernorm(xpre, x2n, False)
                return x2n

            def stage_x2T(q, x2n):
                t = x2t_p.tile([P, DT, P], bf16, tag="x2t", name=f"x2T{q}")
                transpose8(x2n, t, f"x2T{q}")
                x2T[q] = t

            # ---- vocab chunk machinery ----
            covered = [set() for _ in range(NCHUNK)]
            FL_CAP = 16
            fl_state = {"pending": [], "next": 0, "ready": set()}

            def chunk_width(c):
                return CN if c < NFULL else CLAST

            def load_chunk(c, qeng):
                w = chunk_width(c)
                t = wp_p.tile([P, DT, CN], bf16, tag="wp", name=f"wp{c}")
                src = h_wp[c] if c < NFULL else h_wpl[:]
                qeng.dma_start(out=t[:, :, :w], in_=src)
                bt = bp_p.tile([P, CN], bf16, tag="bp", name=f"bp{c}")
                qeng.dma_start(out=bt[:, :w], in_=bcast(h_bp, w, offset=c * CN))
                return t, bt

            def emit_chunk_qt(c, wt, bt, q, out_eng):
                w = chunk_width(c)
                pm = ps.tile([P, 512], f32, tag="ps", name=f"vo{c}_{q}")
                for k in range(DT):
                    nc.tensor.matmul(pm[:, :w], lhsT=x2T[q][:, k, :],
                                     rhs=wt[:, k, :w],
                                     start=(k == 0), stop=(k == DT - 1))
                osb = osb_p.tile([P, CN], bf16, tag="osb", name=f"os{c}_{q}")
                nc.vector.tensor_tensor(out=osb[:, :w], in0=pm[:, :w],
                                        in1=bt[:, :w], op=ALU.add)
                out_eng.dma_start(
                    out=h_out[q * P:(q + 1) * P, c * CN:c * CN + w],
                    in_=osb[:, :w])

            def fl_prefetch():
                """Keep up to 2 front-load chunks DMA'd ahead (wp ring is 3)."""
                st = fl_state
                while len(st["pending"]) < 2 and st["next"] < FL_CAP:
                    c = st["next"]
                    st["next"] = c + 1
                    wt, bt = load_chunk(c, nc.gpsimd)
                    st["pending"].append((c, wt, bt))

            def fl(budget):
                """Emit up to `budget` (chunk, qt) vocab units as PE filler."""
                st = fl_state
                while budget > 0 and st["pending"]:
                    c, wt, bt = st["pending"][0]
                    todo = sorted(st["ready"] - covered[c])
                    if not todo:
                        st["pending"].pop(0)
                        fl_prefetch()
                        continue
                    emit_chunk_qt(c, wt, bt, todo[0], nc.gpsimd)
                    covered[c].add(todo[0])
                    budget -= 1
                    if not (st["ready"] - covered[c]):
                        st["pending"].pop(0)
                        fl_prefetch()

            # ---- pipelined emission ----
            stage_x0(0); stage_qkv(0); stage_scores(0)
            stage_x0(1); stage_qkv(1)
            stage_av(0); stage_x1T_q2(0); stage_scores2(0)
            fl_prefetch()
            stage_scores(1)
            stage_x0(2); stage_qkv(2)
            x2n = stage_av2(0); stage_x2T(0, x2n)
            fl_state["ready"].add(0)
            stage_av(1); fl(2); stage_x1T_q2(1); stage_scores2(1)
            stage_scores(2); fl(1)
            stage_x0(3); stage_qkv(3)
            x2n = stage_av2(1); stage_x2T(1, x2n)
            fl_state["ready"].add(1)
            stage_av(2); fl(2); stage_x1T_q2(2); stage_scores2(2)
            stage_scores(3); fl(2)
            x2n = stage_av2(2); stage_x2T(2, x2n)
            fl_state["ready"].add(2)
            stage_av(3); fl(3); stage_x1T_q2(3); stage_scores2(3); fl(2)
            x2n = stage_av2(3); stage_x2T(3, x2n)

            # ---- main vocab loop over remaining (chunk, qt) work ----
            eng_cycle = [nc.gpsimd, nc.sync]
            out_cycle = [nc.sync, nc.scalar]
            plan = []
            for c in range(NCHUNK):
                rem = [q for q in range(ST) if q not in covered[c]]
                if rem:
                    plan.append((c, rem))
            # prefetch depth 2
            loaded = {}
            for i, (c, rem) in enumerate(plan):
                if i < 2:
                    loaded[c] = load_chunk(c, eng_cycle[i % 2])
            for i, (c, rem) in enumerate(plan):
                if i + 2 < len(plan):
                    c2 = plan[i + 2][0]
                    loaded[c2] = load_chunk(c2, eng_cycle[i % 2])
                wt, bt = loaded.pop(c)
                for j, q in enumerate(rem):
                    emit_chunk_qt(c, wt, bt, q, out_cycle[j % 2])

    nc.compile()
    return nc


def _tile_sq(w, kt):
    """[K, N] -> [128, K//128, N] contiguous."""
    k, n = w.shape
    assert k == kt * P
    return np.ascontiguousarray(
        w.reshape(kt, P, n).transpose(1, 0, 2)).astype(BF16)


def _prep_inputs(inputs):
    g = lambda name: np.asarray(inputs[name], dtype=np.float32)
    tokens = np.asarray(inputs["tokens"]).astype(np.int32)
    img = g("img_emb")

    # positional encoding (same closed form as the model definition)
    posn = np.arange(S)[:, None].astype(np.float32)
    i = np.arange(0, D, 2).astype(np.float32)
    ang = posn / np.power(10000.0, i / D)
    pos = np.zeros((S, D), dtype=np.float32)
    pos[:, 0::2] = np.sin(ang)
    pos[:, 1::2] = np.cos(ang)

    # fold LN2 affine into the vocab projection: out = n@(g2*Wp) + (b2@Wp+bp)
    wp = g("Wp") * g("g2")[:, None]          # [D, V]
    bp_eff = (g("b2") @ g("Wp") + g("bp")).astype(BF16)
    wp_t = np.ascontiguousarray(
        wp.reshape(DT, P, V).transpose(1, 0, 2)).astype(BF16)  # [P, DT, V]
    wp_main = np.ascontiguousarray(
        wp_t[:, :, :NFULL * CN].reshape(P, DT, NFULL, CN)
        .transpose(2, 0, 1, 3))              # [NFULL, P, DT, CN]
    wp_last = np.ascontiguousarray(wp_t[:, :, NFULL * CN:])  # [P, DT, CLAST]

    def bias_tiled(b):
        return np.ascontiguousarray(b.reshape(DT, P).T).astype(np.float32)

    shared = {
        "table": g("emb_table").astype(BF16),
        "pos": pos.astype(BF16),
        "wq1": _tile_sq(g("Wq1") * SCALE, DT),
        "wk1": _tile_sq(g("Wk1"), DT),
        "wv1": _tile_sq(g("Wv1"), DT),
        "wq2": _tile_sq(g("Wq2") * SCALE, DT),
        "wk2": _tile_sq(g("Wk2"), DIT),
        "wv2": _tile_sq(g("Wv2"), DIT),
        "wp": wp_main,
        "wpl": wp_last,
        "bq1": bias_tiled(g("bq1") * SCALE),
        "bk1": bias_tiled(g("bk1")),
        "bq2": bias_tiled(g("bq2") * SCALE),
        "bk2": bias_tiled(g("bk2")),
        "bv1": g("bv1").astype(BF16),
        "bv2": g("bv2").astype(BF16),
        "bp": bp_eff,
        "g1": g("g1").astype(BF16), "b1": g("b1").astype(BF16),
    }
    in_maps = []
    for c in range(N_CORES):
        m = dict(shared)
        m["tok"] = np.ascontiguousarray(tokens[c])
        m["img_t"] = np.ascontiguousarray(
            img[c].T.reshape(DIT, P, NI).transpose(1, 0, 2)).astype(BF16)
        in_maps.append(m)
    return in_maps


def _ensure_axon_hooks():
    """bass_utils imports antenv.axon_hooks when BASS_TRACE is set; stub it
    if the module is absent so tracing degrades instead of crashing."""
    try:
        import antenv.axon_hooks  # noqa: F401
    except ImportError:
        import types
        mod = types.ModuleType("antenv.axon_hooks")
        mod.get_axon_ntff_profile_hook = lambda: None
        mod.set_axon_ntff_profile_hook = lambda h: None
        sys.modules["antenv.axon_hooks"] = mod


def kernel(**inputs):
    global LAST_RESULTS
    _ensure_axon_hooks()
    from concourse.bass_utils import run_bass_kernel_spmd

    if "nc" not in _CACHE:
        _CACHE["nc"] = _build_program()
    nc = _CACHE["nc"]

    in_maps = _prep_inputs(inputs)
    res = run_bass_kernel_spmd(nc, in_maps, core_ids=list(range(N_CORES)))
    LAST_RESULTS = res
    out = np.stack([res.results[c]["out"].astype(np.float32)
                    for c in range(N_CORES)])
    return out


# revision 12
# speedup vs baseline: 1.0566x; 1.0566x over previous
"""Trainium2 Bass kernel for an 8-batch image-conditioned decoder layer.

Strategy: pure data-parallel over the batch — core c computes batch element c
end-to-end.  v3: batched wide attention (512-wide matmuls), exact 32000-wide
vocab projection (63 chunks, no pad), g2/b2 folded into Wp/bp on the host,
in-place PSUM masking, K2/V2 PE warmup, latency-critical DMAs prioritized,
per-qt x2T with vocab chunks front-loaded as PE filler in the cross-attn tail.

All matmuls run in bf16 with fp32 PSUM accumulation.
"""

import os
import sys

for _p in ("/opt/trn_rl_repo", "/root/.axon_site/_ro/trn_rl_repo"):
    if os.path.isdir(_p) and _p not in sys.path:
        sys.path.append(_p)

import numpy as np
import ml_dtypes

BF16 = ml_dtypes.bfloat16

# Problem dims (hardcoded per spec)
V, D, DI, S, B, NI = 32000, 1024, 768, 512, 8, 197
EPS = 1e-5
P = 128
ST = S // P          # 4 seq tiles
DT = D // P          # 8 model-dim tiles
DIT = DI // P        # 6 image-dim tiles
NI2 = NI - P         # 69 rows in second image tile
CN = 512             # vocab chunk width
NFULL = V // CN      # 62 full chunks
CLAST = V - NFULL * CN   # 256
NCHUNK = NFULL + 1   # 63
N_CORES = 8
SCALE = 1.0 / float(np.sqrt(np.float32(D)))

_CACHE = {}
LAST_RESULTS = None


def _build_program():
    import concourse.bacc as bacc
    import concourse.bass as bass
    import concourse.mybir as mybir
    from concourse.masks import make_identity
    from concourse.tile import TileContext

    f32 = mybir.dt.float32
    bf16 = mybir.dt.bfloat16
    i32 = mybir.dt.int32
    X = mybir.AxisListType.X
    ALU = mybir.AluOpType
    ACT_F = mybir.ActivationFunctionType

    nc = bacc.Bacc("TRN2", target_bir_lowering=False, debug=False,
                   num_devices=N_CORES)

    # ---- I/O ----
    h_tok = nc.dram_tensor("tok", [S], i32, kind="ExternalInput")
    h_table = nc.dram_tensor("table", [V, D], bf16, kind="ExternalInput")
    h_pos = nc.dram_tensor("pos", [S, D], bf16, kind="ExternalInput")
    h_img = nc.dram_tensor("img_t", [P, DIT, NI], bf16, kind="ExternalInput")
    h_wq1 = nc.dram_tensor("wq1", [P, DT, D], bf16, kind="ExternalInput")
    h_wk1 = nc.dram_tensor("wk1", [P, DT, D], bf16, kind="ExternalInput")
    h_wv1 = nc.dram_tensor("wv1", [P, DT, D], bf16, kind="ExternalInput")
    h_wq2 = nc.dram_tensor("wq2", [P, DT, D], bf16, kind="ExternalInput")
    h_wk2 = nc.dram_tensor("wk2", [P, DIT, D], bf16, kind="ExternalInput")
    h_wv2 = nc.dram_tensor("wv2", [P, DIT, D], bf16, kind="ExternalInput")
    h_wp = nc.dram_tensor("wp", [NFULL, P, DT, CN], bf16, kind="ExternalInput")
    h_wpl = nc.dram_tensor("wpl", [P, DT, CLAST], bf16, kind="ExternalInput")
    h_bq1 = nc.dram_tensor("bq1", [P, DT], f32, kind="ExternalInput")
    h_bk1 = nc.dram_tensor("bk1", [P, DT], f32, kind="ExternalInput")
    h_bq2 = nc.dram_tensor("bq2", [P, DT], f32, kind="ExternalInput")
    h_bk2 = nc.dram_tensor("bk2", [P, DT], f32, kind="ExternalInput")
    h_bv1 = nc.dram_tensor("bv1", [D], bf16, kind="ExternalInput")
    h_bv2 = nc.dram_tensor("bv2", [D], bf16, kind="ExternalInput")
    h_bp = nc.dram_tensor("bp", [V], bf16, kind="ExternalInput")
    h_g1 = nc.dram_tensor("g1", [D], bf16, kind="ExternalInput")
    h_b1 = nc.dram_tensor("b1", [D], bf16, kind="ExternalInput")
    h_out = nc.dram_tensor("out", [S, V], bf16, kind="ExternalOutput")

    def bcast(handle, n, offset=0):
        ap = handle[:]
        return bass.AP(tensor=ap.tensor, offset=offset, ap=[[0, P], [1, n]])

    with TileContext(nc) as tc:
        import contextlib
        ctx = contextlib.ExitStack()
        with ctx:
            const = ctx.enter_context(tc.tile_pool(name="const", bufs=1))
            posp = ctx.enter_context(tc.tile_pool(name="posp", bufs=4))
            x0b_p = ctx.enter_context(tc.tile_pool(name="x0b", bufs=4))
            xt_p = ctx.enter_context(tc.tile_pool(name="xt", bufs=2))
            qk_p = ctx.enter_context(tc.tile_pool(name="qk", bufs=2))
            v_p = ctx.enter_context(tc.tile_pool(name="vp", bufs=1))
            k2t_p = ctx.enter_context(tc.tile_pool(name="k2t", bufs=1))
            v2t_p = ctx.enter_context(tc.tile_pool(name="v2t", bufs=1))
            pb_p = ctx.enter_context(tc.tile_pool(name="pb", bufs=4))
            pt_p = ctx.enter_context(tc.tile_pool(name="pt", bufs=2))
            x1b_p = ctx.enter_context(tc.tile_pool(name="x1b", bufs=1))
            p2b_p = ctx.enter_context(tc.tile_pool(name="p2b", bufs=2))
            pt2_p = ctx.enter_context(tc.tile_pool(name="pt2", bufs=2))
            x2n_p = ctx.enter_context(tc.tile_pool(name="x2n", bufs=2))
            x2t_p = ctx.enter_context(tc.tile_pool(name="x2t", bufs=4))
            xpre_p = ctx.enter_context(tc.tile_pool(name="xpre", bufs=2))
            stat_p = ctx.enter_context(tc.tile_pool(name="stat", bufs=4))
            wts_p = ctx.enter_context(tc.tile_pool(name="wts", bufs=4))
            wp_p = ctx.enter_context(tc.tile_pool(name="wpp", bufs=3))
            bp_p = ctx.enter_context(tc.tile_pool(name="bpp", bufs=3))
            osb_p = ctx.enter_context(tc.tile_pool(name="osb", bufs=5))
            ps = ctx.enter_context(tc.tile_pool(name="ps", bufs=6,
                                                space="PSUM"))
            ps2 = ctx.enter_context(tc.tile_pool(name="ps2", bufs=2,
                                                 space="PSUM"))

            # ---- t=0 DMA kickoff: latency-critical small transfers first ----
            tok_sb = const.tile([P, ST], i32)
            nc.sync.dma_start(out=tok_sb,
                              in_=h_tok[:].rearrange("(a p) -> p a", p=P))
            img_sb = const.tile([P, DIT, NI], bf16)
            nc.scalar.dma_start(out=img_sb, in_=h_img[:])
            pos_sb = []
            for q in range(ST):
                t = posp.tile([P, D], bf16, tag="pos", name=f"pos{q}")
                eng = nc.sync if q < 2 else nc.scalar
                eng.dma_start(out=t, in_=h_pos[q * P:(q + 1) * P, :])
                pos_sb.append(t)
            # embedding gather (gpsimd; needs tok only)
            x0b = [x0b_p.tile([P, D], bf16, tag="x0b", name=f"x0b{q}")
                   for q in range(ST)]
            for q in range(ST):
                nc.gpsimd.indirect_dma_start(
                    out=x0b[q], out_offset=None, in_=h_table[:],
                    in_offset=bass.IndirectOffsetOnAxis(ap=tok_sb[:, q:q + 1],
                                                        axis=0))
            wk2_sb = wts_p.tile([P, DIT, D], bf16, tag="wts", name="wk2")
            nc.scalar.dma_start(out=wk2_sb, in_=h_wk2[:])
            wv2_sb = wts_p.tile([P, DIT, D], bf16, tag="wts", name="wv2")
            nc.scalar.dma_start(out=wv2_sb, in_=h_wv2[:])

            epst = const.tile([P, 1], f32)
            nc.vector.memset(epst, EPS)
            bq1s = const.tile([P, DT], f32)
            bk1s = const.tile([P, DT], f32)
            bq2s = const.tile([P, DT], f32)
            bk2s = const.tile([P, DT], f32)
            for t, h in ((bk2s, h_bk2), (bq1s, h_bq1), (bk1s, h_bk1),
                         (bq2s, h_bq2)):
                nc.sync.dma_start(out=t, in_=h[:])
            g1b = const.tile([P, D], bf16)
            b1b = const.tile([P, D], bf16)
            bv1b = const.tile([P, D], bf16)
            bv2b = const.tile([P, D], bf16)
            for t, h in ((bv2b, h_bv2), (bv1b, h_bv1), (g1b, h_g1),
                         (b1b, h_b1)):
                nc.sync.dma_start(out=t, in_=bcast(h, D))

            ident = const.tile([P, P], bf16)
            make_identity(nc, ident)
            trimask = const.tile([P, P], f32)
            nc.gpsimd.memset(trimask, 0.0)
            nc.gpsimd.affine_select(
                out=trimask, in_=trimask, compare_op=ALU.is_ge, fill=-1e10,
                base=0, pattern=[[-1, P]], channel_multiplier=1)

            # big weights after the latency-critical loads
            wq1_sb = wts_p.tile([P, DT, D], bf16, tag="wts", name="wq1")
            nc.sync.dma_start(out=wq1_sb, in_=h_wq1[:])
            wk1_sb = wts_p.tile([P, DT, D], bf16, tag="wts", name="wk1")
            nc.scalar.dma_start(out=wk1_sb, in_=h_wk1[:])
            # x0 = gather + pos (vector; ready early)
            for q in range(ST):
                nc.vector.tensor_tensor(out=x0b[q], in0=x0b[q],
                                        in1=pos_sb[q], op=ALU.add)

            def transpose8(src, dst_ap_fn):
                """src [P, D] bf16 (seq-part) -> 8 d-part blocks via PE."""
                for db in range(DT):
                    tp = ps2.tile([P, 1024], bf16, tag="ps2", name="tp")
                    nc.tensor.transpose(out=tp[:, :P],
                                        in_=src[:, db * P:(db + 1) * P],
                                        identity=ident)
                    nc.vector.tensor_copy(out=dst_ap_fn(db), in_=tp[:, :P])

            # ---- x0T transposes first (inputs ready ~5us in) ----
            x0T = xt_p.tile([P, DT, S], bf16, tag="xt", name="x0T")
            for q in range(ST):
                transpose8(x0b[q], lambda db, q=q: x0T[:, db, q * P:(q + 1) * P])

            # ---- PE warmup: K2T / V2t (depend only on img weights) ----
            K2T = k2t_p.tile([P, DT, NI], bf16, tag="k2t")
            for m in range(DT):
                pm = ps.tile([P, 512], f32, tag="ps", name="k2ps")
                for k in range(DIT):
                    nc.tensor.matmul(pm[:, :NI],
                                     lhsT=wk2_sb[:, k, m * P:(m + 1) * P],
                                     rhs=img_sb[:, k, :],
                                     start=(k == 0), stop=(k == DIT - 1))
                nc.scalar.activation(out=K2T[:, m, :], in_=pm[:, :NI],
                                     func=ACT_F.Identity,
                                     bias=bk2s[:, m:m + 1], scale=1.0)
            # wv1 reuses wk2's pool slot; emit after K2T (wk2's readers) exist
            wv1_sb = wts_p.tile([P, DT, D], bf16, tag="wts", name="wv1")
            nc.sync.dma_start(out=wv1_sb, in_=h_wv1[:])
            V2t = v2t_p.tile([P, 2, D], bf16, tag="v2t")
            for a in range(2):
                pa = P if a == 0 else NI2
                for nh in range(2):
                    pm = ps.tile([P, 512], f32, tag="ps", name="v2ps")
                    for k in range(DIT):
                        nc.tensor.matmul(
                            pm[:pa, :], lhsT=img_sb[:, k, a * P:a * P + pa],
                            rhs=wv2_sb[:, k, nh * 512:(nh + 1) * 512],
                            start=(k == 0), stop=(k == DIT - 1))
                    nc.vector.tensor_tensor(
                        out=V2t[:pa, a, nh * 512:(nh + 1) * 512], in0=pm[:pa, :],
                        in1=bv2b[:pa, nh * 512:(nh + 1) * 512], op=ALU.add)

            # wq2 reuses wv2's pool slot; emit after V2t (wv2's readers)
            wq2_sb = wts_p.tile([P, DT, D], bf16, tag="wts", name="wq2")
            nc.scalar.dma_start(out=wq2_sb, in_=h_wq2[:])

            # ---- projections (full-seq, 512-wide) ----
            def proj_T(w_sb, b_sb, rhsT, pool, name):
                """out[P, DT, S] bf16 = (W.T @ x.T) + b, d-partition."""
                o = pool.tile([P, DT, S], bf16, tag="qk", name=name)
                for m in range(DT):
                    pm = ps.tile([P, 512], f32, tag="ps", name="pm")
                    for k in range(DT):
                        nc.tensor.matmul(pm, lhsT=w_sb[:, k, m * P:(m + 1) * P],
                                         rhs=rhsT[:, k, :],
                                         start=(k == 0), stop=(k == DT - 1))
                    nc.scalar.activation(out=o[:, m, :], in_=pm,
                                         func=ACT_F.Identity,
                                         bias=b_sb[:, m:m + 1], scale=1.0)
                return o

            QT = proj_T(wq1_sb, bq1s, x0T, qk_p, "qt")
            KT = proj_T(wk1_sb, bk1s, x0T, qk_p, "kt")

            Vt = v_p.tile([P, ST, D], bf16, tag="v", name="vt")
            for a in range(ST):
                for nh in range(2):
                    pm = ps.tile([P, 512], f32, tag="ps", name="vps")
                    for k in range(DT):
                        nc.tensor.matmul(
                            pm, lhsT=x0T[:, k, a * P:(a + 1) * P],
                            rhs=wv1_sb[:, k, nh * 512:(nh + 1) * 512],
                            start=(k == 0), stop=(k == DT - 1))
                    nc.vector.tensor_tensor(
                        out=Vt[:, a, nh * 512:(nh + 1) * 512], in0=pm,
                        in1=bv1b[:, nh * 512:(nh + 1) * 512], op=ALU.add)

            # ---- causal self-attention: scores + softmax, then AV + LN1 ----
            rinv1 = const.tile([P, ST], f32)
            rinv2 = const.tile([P, ST], f32)
            Pbs = []
            for qt in range(ST):
                width = (qt + 1) * P
                pm = ps.tile([P, 512], f32, tag="ps", name=f"sc{qt}")
                for k in range(DT):
                    nc.tensor.matmul(pm[:, :width],
                                     lhsT=QT[:, k, qt * P:(qt + 1) * P],
                                     rhs=KT[:, k, :width],
                                     start=(k == 0), stop=(k == DT - 1))
                # in-place causal mask on the diagonal block (PSUM RMW)
                nc.vector.tensor_tensor(out=pm[:, qt * P:width],
                                        in0=pm[:, qt * P:width], in1=trimask,
                                        op=ALU.add)
                nmax = stat_p.tile([P, 1], f32, tag="nmax")
                nc.vector.reduce_max(nmax, pm[:, :width], axis=X, negate=True)
                Pb = pb_p.tile([P, 512], bf16, tag="pb", name=f"pb{qt}")
                rsum = stat_p.tile([P, 1], f32, tag="rsum")
                nc.scalar.activation(out=Pb[:, :width], in_=pm[:, :width],
                                     func=ACT_F.Exp, bias=nmax, scale=1.0,
                                     accum_out=rsum)
                nc.vector.reciprocal(out=rinv1[:, qt:qt + 1], in_=rsum)
                Pbs.append(Pb)

            def layernorm(xpre, out_sl, affine):
                """xpre [P, D] f32 -> out_sl [P, D] bf16."""
                stats = stat_p.tile([P, 2, 6], f32, tag="bnst")
                for sg in range(2):
                    nc.vector.bn_stats(out=stats[:, sg, :],
                                       in_=xpre[:, sg * 512:(sg + 1) * 512])
                mv = stat_p.tile([P, 2], f32, tag="bnmv")
                nc.vector.bn_aggr(out=mv, in_=stats)
                rstd = stat_p.tile([P, 1], f32, tag="rstd")
                nc.scalar.activation(out=rstd, in_=mv[:, 1:2], func=ACT_F.Sqrt,
                                     bias=epst, scale=1.0)
                nc.vector.reciprocal(out=rstd, in_=rstd)
                nmr = stat_p.tile([P, 1], f32, tag="nmr")
                nc.vector.scalar_tensor_tensor(out=nmr, in0=mv[:, 0:1],
                                               scalar=-1.0, in1=rstd,
                                               op0=ALU.mult, op1=ALU.mult)
                if affine:
                    nc.scalar.activation(out=xpre, in_=xpre,
                                         func=ACT_F.Identity,
                                         bias=nmr, scale=rstd)
                    nc.vector.tensor_tensor(out=xpre, in0=xpre, in1=g1b,
                                            op=ALU.mult)
                    nc.vector.tensor_tensor(out=out_sl, in0=xpre, in1=b1b,
                                            op=ALU.add)
                else:
                    nc.scalar.activation(out=out_sl, in_=xpre,
                                         func=ACT_F.Identity,
                                         bias=nmr, scale=rstd)

            x1b = x1b_p.tile([P, ST, D], bf16, tag="x1b")
            for qt in range(ST):
                PT = pt_p.tile([P, ST, P], bf16, tag="pt", name=f"pt{qt}")
                for kt in range(qt + 1):
                    tp = ps2.tile([P, 1024], bf16, tag="ps2", name="tp")
                    nc.tensor.transpose(out=tp[:, :P],
                                        in_=Pbs[qt][:, kt * P:(kt + 1) * P],
                                        identity=ident)
                    nc.vector.tensor_copy(out=PT[:, kt, :], in_=tp[:, :P])
                xpre = xpre_p.tile([P, D], f32, tag="xpre")
                for nh in range(2):
                    pm = ps.tile([P, 512], f32, tag="ps", name="avps")
                    for kt in range(qt + 1):
                        nc.tensor.matmul(
                            pm, lhsT=PT[:, kt, :],
                            rhs=Vt[:, kt, nh * 512:(nh + 1) * 512],
                            start=(kt == 0), stop=(kt == qt))
                    nc.vector.scalar_tensor_tensor(
                        out=xpre[:, nh * 512:(nh + 1) * 512], in0=pm,
                        scalar=rinv1[:, qt:qt + 1],
                        in1=x0b[qt][:, nh * 512:(nh + 1) * 512],
                        op0=ALU.mult, op1=ALU.add)
                layernorm(xpre, x1b[:, qt, :], True)

            # ---- cross attention ----
            x1T = xt_p.tile([P, DT, S], bf16, tag="xt", name="x1T")
            for q in range(ST):
                transpose8(x1b[:, q, :],
                           lambda db, q=q: x1T[:, db, q * P:(q + 1) * P])
            Q2T = proj_T(wq2_sb, bq2s, x1T, qk_p, "q2t")

            P2bs = []
            for qt in range(ST):
                pm = ps.tile([P, 512], f32, tag="ps", name=f"sc2{qt}")
                for k in range(DT):
                    nc.tensor.matmul(pm[:, :NI],
                                     lhsT=Q2T[:, k, qt * P:(qt + 1) * P],
                                     rhs=K2T[:, k, :],
                                     start=(k == 0), stop=(k == DT - 1))
                nmax = stat_p.tile([P, 1], f32, tag="nmax")
                nc.vector.reduce_max(nmax, pm[:, :NI], axis=X, negate=True)
                P2b = p2b_p.tile([P, NI], bf16, tag="p2b", name=f"p2b{qt}")
                rsum = stat_p.tile([P, 1], f32, tag="rsum")
                nc.scalar.activation(out=P2b, in_=pm[:, :NI],
                                     func=ACT_F.Exp, bias=nmax, scale=1.0,
                                     accum_out=rsum)
                nc.vector.reciprocal(out=rinv2[:, qt:qt + 1], in_=rsum)
                P2bs.append(P2b)

            # ---- vocab chunk machinery ----
            covered = [set() for _ in range(NCHUNK)]
            FL_CAP = 20
            fl_state = {"pending": [], "next": 0, "ready": set()}
            x2T = [None] * ST

            def chunk_width(c):
                return CN if c < NFULL else CLAST

            def load_chunk(c, qeng):
                w = chunk_width(c)
                t = wp_p.tile([P, DT, CN], bf16, tag="wp", name=f"wp{c}")
                src = h_wp[c] if c < NFULL else h_wpl[:]
                qeng.dma_start(out=t[:, :, :w], in_=src)
                bt = bp_p.tile([P, CN], bf16, tag="bp", name=f"bp{c}")
                qeng.dma_start(out=bt[:, :w], in_=bcast(h_bp, w, offset=c * CN))
                return t, bt

            def emit_chunk_qt(c, wt, bt, q, out_eng):
                w = chunk_width(c)
                pm = ps.tile([P, 512], f32, tag="ps", name=f"vo{c}_{q}")
                for k in range(DT):
                    nc.tensor.matmul(pm[:, :w], lhsT=x2T[q][:, k, :],
                                     rhs=wt[:, k, :w],
                                     start=(k == 0), stop=(k == DT - 1))
                osb = osb_p.tile([P, CN], bf16, tag="osb", name=f"os{c}_{q}")
                nc.vector.tensor_tensor(out=osb[:, :w], in0=pm[:, :w],
                                        in1=bt[:, :w], op=ALU.add)
                out_eng.dma_start(
                    out=h_out[q * P:(q + 1) * P, c * CN:c * CN + w],
                    in_=osb[:, :w])

            def fl_prefetch():
                """Keep up to 2 front-load chunks DMA'd ahead (wp ring is 3)."""
                st = fl_state
                while len(st["pending"]) < 2 and st["next"] < FL_CAP:
                    c = st["next"]
                    st["next"] = c + 1
                    wt, bt = load_chunk(c, nc.gpsimd)
                    st["pending"].append((c, wt, bt))

            def fl(budget):
                """Emit up to `budget` (chunk, qt) vocab units as PE filler."""
                st = fl_state
                while budget > 0 and st["pending"]:
                    c, wt, bt = st["pending"][0]
                    todo = sorted(st["ready"] - covered[c])
                    if not todo:
                        st["pending"].pop(0)
                        fl_prefetch()
                        continue
                    emit_chunk_qt(c, wt, bt, todo[0], nc.gpsimd)
                    covered[c].add(todo[0])
                    budget -= 1
                    if not (st["ready"] - covered[c]):
                        st["pending"].pop(0)
                        fl_prefetch()

            fl_prefetch()

            # ---- cross-attn tail: per-qt AV2 + LN2 + x2T, vocab filler ----
            for qt in range(ST):
                PT2 = pt2_p.tile([P, 2, P], bf16, tag="pt2", name=f"pt2{qt}")
                tp = ps2.tile([P, 1024], bf16, tag="ps2", name="tp")
                nc.tensor.transpose(out=tp[:, :P], in_=P2bs[qt][:, :P],
                                    identity=ident)
                nc.vector.tensor_copy(out=PT2[:, 0, :], in_=tp[:, :P])
                tp = ps2.tile([P, 1024], bf16, tag="ps2", name="tp")
                nc.tensor.transpose(out=tp[:NI2, :P], in_=P2bs[qt][:, P:NI],
                                    identity=ident)
                nc.vector.tensor_copy(out=PT2[:NI2, 1, :], in_=tp[:NI2, :P])
                xpre = xpre_p.tile([P, D], f32, tag="xpre")
                for nh in range(2):
                    pm = ps.tile([P, 512], f32, tag="ps", name="av2ps")
                    nc.tensor.matmul(pm, lhsT=PT2[:, 0, :],
                                     rhs=V2t[:, 0, nh * 512:(nh + 1) * 512],
                                     start=True, stop=False)
                    nc.tensor.matmul(pm, lhsT=PT2[:NI2, 1, :],
                                     rhs=V2t[:NI2, 1, nh * 512:(nh + 1) * 512],
                                     start=False, stop=True)
                    nc.vector.scalar_tensor_tensor(
                        out=xpre[:, nh * 512:(nh + 1) * 512], in0=pm,
                        scalar=rinv2[:, qt:qt + 1],
                        in1=x1b[:, qt, nh * 512:(nh + 1) * 512],
                        op0=ALU.mult, op1=ALU.add)
                x2n = x2n_p.tile([P, D], bf16, tag="x2n", name=f"x2n{qt}")
                layernorm(xpre, x2n, False)
                t = x2t_p.tile([P, DT, P], bf16, tag="x2t", name=f"x2T{qt}")
                transpose8(x2n, lambda db: t[:, db, :])
                x2T[qt] = t
                fl_state["ready"].add(qt)
                fl((qt + 1) * 2)

            # ---- main vocab loop over remaining (chunk, qt) work ----
            eng_cycle = [nc.gpsimd, nc.sync]
            out_cycle = [nc.sync, nc.scalar]
            plan = []
            for c in range(NCHUNK):
                rem = [q for q in range(ST) if q not in covered[c]]
                if rem:
                    plan.append((c, rem))
            loaded = {}
            for i, (c, rem) in enumerate(plan):
                if i < 2:
                    loaded[c] = load_chunk(c, eng_cycle[i % 2])
            for i, (c, rem) in enumerate(plan):
                if i + 2 < len(plan):
                    c2 = plan[i + 2][0]
                    loaded[c2] = load_chunk(c2, eng_cycle[i % 2])
                wt, bt = loaded.pop(c)
                for j, q in enumerate(rem):
                    emit_chunk_qt(c, wt, bt, q, out_cycle[j % 2])

    nc.compile()
    return nc


def _tile_sq(w, kt):
    """[K, N] -> [128, K//128, N] contiguous."""
    k, n = w.shape
    assert k == kt * P
    return np.ascontiguousarray(
        w.reshape(kt, P, n).transpose(1, 0, 2)).astype(BF16)


def _prep_inputs(inputs):
    g = lambda name: np.asarray(inputs[name], dtype=np.float32)
    tokens = np.asarray(inputs["tokens"]).astype(np.int32)
    img = g("img_emb")

    # positional encoding (same closed form as the model definition)
    posn = np.arange(S)[:, None].astype(np.float32)
    i = np.arange(0, D, 2).astype(np.float32)
    ang = posn / np.power(10000.0, i / D)
    pos = np.zeros((S, D), dtype=np.float32)
    pos[:, 0::2] = np.sin(ang)
    pos[:, 1::2] = np.cos(ang)

    # fold LN2 affine into the vocab projection: out = n@(g2*Wp) + (b2@Wp+bp)
    wp = g("Wp") * g("g2")[:, None]          # [D, V]
    bp_eff = (g("b2") @ g("Wp") + g("bp")).astype(BF16)
    wp_t = np.ascontiguousarray(
        wp.reshape(DT, P, V).transpose(1, 0, 2)).astype(BF16)  # [P, DT, V]
    wp_main = np.ascontiguousarray(
        wp_t[:, :, :NFULL * CN].reshape(P, DT, NFULL, CN)
        .transpose(2, 0, 1, 3))              # [NFULL, P, DT, CN]
    wp_last = np.ascontiguousarray(wp_t[:, :, NFULL * CN:])  # [P, DT, CLAST]

    def bias_tiled(b):
        return np.ascontiguousarray(b.reshape(DT, P).T).astype(np.float32)

    shared = {
        "table": g("emb_table").astype(BF16),
        "pos": pos.astype(BF16),
        "wq1": _tile_sq(g("Wq1") * SCALE, DT),
        "wk1": _tile_sq(g("Wk1"), DT),
        "wv1": _tile_sq(g("Wv1"), DT),
        "wq2": _tile_sq(g("Wq2") * SCALE, DT),
        "wk2": _tile_sq(g("Wk2"), DIT),
        "wv2": _tile_sq(g("Wv2"), DIT),
        "wp": wp_main,
        "wpl": wp_last,
        "bq1": bias_tiled(g("bq1") * SCALE),
        "bk1": bias_tiled(g("bk1")),
        "bq2": bias_tiled(g("bq2") * SCALE),
        "bk2": bias_tiled(g("bk2")),
        "bv1": g("bv1").astype(BF16),
        "bv2": g("bv2").astype(BF16),
        "bp": bp_eff,
        "g1": g("g1").astype(BF16), "b1": g("b1").astype(BF16),
    }
    in_maps = []
    for c in range(N_CORES):
        m = dict(shared)
        m["tok"] = np.ascontiguousarray(tokens[c])
        m["img_t"] = np.ascontiguousarray(
            img[c].T.reshape(DIT, P, NI).transpose(1, 0, 2)).astype(BF16)
        in_maps.append(m)
    return in_maps


def _ensure_axon_hooks():
    """bass_utils imports antenv.axon_hooks when BASS_TRACE is set; stub it
    if the module is absent so tracing degrades instead of crashing."""
    try:
        import antenv.axon_hooks  # noqa: F401
    except ImportError:
        import types
        mod = types.ModuleType("antenv.axon_hooks")
        mod.get_axon_ntff_profile_hook = lambda: None
        mod.set_axon_ntff_profile_hook = lambda h: None
        sys.modules["antenv.axon_hooks"] = mod


def kernel(**inputs):
    global LAST_RESULTS
    _ensure_axon_hooks()
    from concourse.bass_utils import run_bass_kernel_spmd

    if "nc" not in _CACHE:
        _CACHE["nc"] = _build_program()
    nc = _CACHE["nc"]

    in_maps = _prep_inputs(inputs)
    res = run_bass_kernel_spmd(nc, in_maps, core_ids=list(range(N_CORES)))
    LAST_RESULTS = res
    out = np.stack([res.results[c]["out"].astype(np.float32)
                    for c in range(N_CORES)])
    return out
